# revision 10
# baseline (speedup 1.0000x reference)
"""Trainium2 Bass kernel for nn_MultiHeadAttention (B=4, SQ=SK=1024, D=1024,
H=16, DK=DV=64), sharded over 8 NeuronCores as (batch, head-half).

Each core computes one batch's attention for 8 heads:
  - attns_half[h, q, k]  (bf16, upcast on host)
  - out_partial[s, D]    (f32; host sums the 2 cores of each batch + b_proj)

Layouts on chip (partition dim first):
  qT/kT/vT   : [D, S] (host-pretransposed, bf16)
  qTs/kTs[p] : [128 = dk(h0)|dk(h1), S]   per head-pair p
  v_aug[st]  : [128 s, 4 pairs, 2 heads, 65 = 64 dv | 1 ones]
  scores     : PE K=64 matmuls, both [q,k] and [k,q] orientations
  softmax    : exp on ScalarE (scale=1/32 folded in, no max subtraction --
               scores are ~N(0, 0.22), max |score/32| < 3, exp safe),
               rowsum via accum_out ([q,k]) / ones-column of v_aug ([k,q])
"""

import numpy as np
import ml_dtypes

H, D, DK, DV = 16, 1024, 64, 64
B, SQ, SK = 4, 1024, 1024
P = 128
S = 1024
HHALF = 8  # heads per core
NPAIR = 4  # head pairs per core
SCALE = 1.0 / 32.0  # 1/sqrt(D)

_CACHE = {}


def _build_nc():
    from contextlib import ExitStack

    import concourse.tile as tile
    from concourse import bacc, mybir

    BF16 = mybir.dt.bfloat16
    F32 = mybir.dt.float32
    Exp = mybir.ActivationFunctionType.Exp

    nc = bacc.Bacc("TRN2", target_bir_lowering=False, debug=False)

    qT_d = nc.declare_dram_parameter("qT", [D, S], BF16, isOutput=False)
    kT_d = nc.declare_dram_parameter("kT", [D, S], BF16, isOutput=False)
    vT_d = nc.declare_dram_parameter("vT", [D, S], BF16, isOutput=False)
    wq_d = nc.declare_dram_parameter("wq", [D, HHALF * DK], BF16, isOutput=False)
    wk_d = nc.declare_dram_parameter("wk", [D, HHALF * DK], BF16, isOutput=False)
    wv_d = nc.declare_dram_parameter("wv", [D, HHALF * DV], BF16, isOutput=False)
    wp_d = nc.declare_dram_parameter("wprojT", [HHALF * DV, D], BF16, isOutput=False)
    attns_d = nc.declare_dram_parameter("attns", [HHALF, SQ, SK], BF16, isOutput=True)
    out_d = nc.declare_dram_parameter("out", [SQ, D], F32, isOutput=True)

    with ExitStack() as ctx:
        tc = ctx.enter_context(tile.TileContext(nc))
        ins = ctx.enter_context(tc.tile_pool(name="ins", bufs=1))
        proj = ctx.enter_context(tc.tile_pool(name="proj", bufs=1))
        work = ctx.enter_context(tc.tile_pool(name="work", bufs=3))
        small = ctx.enter_context(tc.tile_pool(name="small", bufs=4))
        norm = ctx.enter_context(tc.tile_pool(name="norm", bufs=2))
        # PSUM budget: 8 banks of [128, 512]f32.
        #   ps_a ([128,1024] x bufs=2) = 4 banks  (scoresT for PV + projections)
        #   ps_b ([128,1024] x bufs=1) = 2 banks  (scores for attns output)
        #   pacc ([128,1024] x bufs=1) = 2 banks  (PV accumulator)
        psum = ctx.enter_context(tc.tile_pool(name="psum", bufs=2, space="PSUM"))
        psum_b = ctx.enter_context(tc.tile_pool(name="psum_b", bufs=1, space="PSUM"))
        pacc = ctx.enter_context(tc.tile_pool(name="pacc", bufs=1, space="PSUM"))

        # ---------------- input loads ----------------
        qT_in = ins.tile([P, 8, S], BF16)
        nc.sync.dma_start(out=qT_in, in_=qT_d.rearrange("(c p) s -> p c s", p=P))
        kT_in = ins.tile([P, 8, S], BF16)
        nc.sync.dma_start(out=kT_in, in_=kT_d.rearrange("(c p) s -> p c s", p=P))
        vT_in = ins.tile([P, 8, S], BF16)
        nc.sync.dma_start(out=vT_in, in_=vT_d.rearrange("(c p) s -> p c s", p=P))
        wq_in = ins.tile([P, 8, HHALF * DK], BF16)
        nc.sync.dma_start(out=wq_in, in_=wq_d.rearrange("(c p) m -> p c m", p=P))
        wk_in = ins.tile([P, 8, HHALF * DK], BF16)
        nc.sync.dma_start(out=wk_in, in_=wk_d.rearrange("(c p) m -> p c m", p=P))
        wv_in = ins.tile([P, 8, HHALF * DV], BF16)
        nc.sync.dma_start(out=wv_in, in_=wv_d.rearrange("(c p) m -> p c m", p=P))
        wp_in = ins.tile([P, NPAIR, D], BF16)
        nc.sync.dma_start(out=wp_in, in_=wp_d.rearrange("(c p) m -> p c m", p=P))

        # ---------------- q/k projections -> per-head zero-padded [dk, s] ----
        # qTz[h] is [128, S]: head 2p's dk rows live at 0:64 (rows 64:128
        # zero), head 2p+1's at 64:128 (rows 0:64 zero). Scores then run as
        # full K=128 matmuls (the zero rows contribute nothing) — keeping the
        # PE array fully active so the HAM clock-gate stays at 8/8.
        qTz = [None] * HHALF
        kTz = [None] * HHALF
        for pair in range(NPAIR):
            for w_in, src_in, dst_list, nm in (
                (wq_in, qT_in, qTz, "q"),
                (wk_in, kT_in, kTz, "k"),
            ):
                ps = psum.tile([P, S], F32, name=f"ps_{nm}{pair}", tag="ps_big")
                for c in range(8):
                    for hf in range(2):
                        nc.tensor.matmul(
                            ps[:, hf * 512 : (hf + 1) * 512],
                            lhsT=w_in[:, c, pair * P : (pair + 1) * P],
                            rhs=src_in[:, c, hf * 512 : (hf + 1) * 512],
                            start=(c == 0),
                            stop=(c == 7),
                        )
                for sub in range(2):
                    h = 2 * pair + sub
                    dst = proj.tile(
                        [P, S], BF16, name=f"{nm}Tz{h}", tag=f"{nm}Tz{h}"
                    )
                    zsl = slice(64 * (1 - sub), 64 * (1 - sub) + 64)
                    hsl = slice(64 * sub, 64 * sub + 64)
                    nc.vector.memset(dst[zsl, :], 0.0)
                    nc.vector.tensor_copy(out=dst[hsl, :], in_=ps[hsl, :])
                    dst_list[h] = dst

        # ---------------- v projection -> [s, head, dv | ones] ----------------
        # v_aug[st] : [128, 4 pairs, 2 heads, 65]; col 64 of each head is 1.0
        v_aug = []
        for st in range(8):
            va = proj.tile([P, NPAIR, 2, DV + 1], BF16, name=f"v_aug{st}", tag=f"v_aug{st}")
            ps = psum.tile([P, HHALF * DV], F32, name=f"ps_v{st}", tag="ps_big")
            for c in range(8):
                nc.tensor.matmul(
                    ps,
                    lhsT=vT_in[:, c, st * P : (st + 1) * P],
                    rhs=wv_in[:, c, :],
                    start=(c == 0),
                    stop=(c == 7),
                )
            ps_v = ps.rearrange("p (pr two d) -> p pr two d", two=2, d=DV)
            nc.vector.tensor_copy(out=va[:, :, :, 0:DV], in_=ps_v)
            nc.vector.memset(va[:, :, :, DV : DV + 1], 1.0)
            v_aug.append(va)

        # ---------------- attention: pairs, A/B row-interleaved ----------------
        # bf16 ones column at partition 64, used to broadcast the PV rowsum
        # (which lands on partition 64 of ps_acc) across partitions 0:64 via
        # a K=1 matmul (compute engines cannot move data across partitions).
        ones_t = proj.tile([P, DV], BF16, name="ones_t", tag="ones_t")
        nc.vector.memset(ones_t, 1.0)
        outTs = [
            proj.tile([P, S], BF16, name=f"outTs{p}", tag=f"outTs{p}")
            for p in range(NPAIR)
        ]
        # Per pair, two stages: head `ha` runs the [k,q] orientation
        # (attnT -> PV) while head `hb` runs [q,k] (normalized attn ->
        # attns output). ha/hb occupy disjoint dk rows (0:64 / 64:128), so
        # their K=64 score matmuls execute concurrently in the PE array.
        for pair in range(NPAIR):
            for stage in range(2):
                sa = stage
                sb = 1 - stage
                ha = 2 * pair + sa
                hb = 2 * pair + sb
                ps_acc = pacc.tile([P, S], F32, name="ps_acc", tag="pacc")
                for t in range(8):
                    ps_a = psum.tile([P, S], F32, name="ps_a", tag="ps_big")
                    ps_b = psum_b.tile([P, S], F32, name="ps_b", tag="ps_bb")
                    for hf in range(2):
                        nc.tensor.matmul(
                            ps_a[:, hf * 512 : (hf + 1) * 512],
                            lhsT=kTz[ha][:, t * P : (t + 1) * P],
                            rhs=qTz[ha][:, hf * 512 : (hf + 1) * 512],
                            start=True,
                            stop=True,
                        )
                        nc.tensor.matmul(
                            ps_b[:, hf * 512 : (hf + 1) * 512],
                            lhsT=qTz[hb][:, t * P : (t + 1) * P],
                            rhs=kTz[hb][:, hf * 512 : (hf + 1) * 512],
                            start=True,
                            stop=True,
                        )
                    attnT = work.tile([P, S], BF16, name="attnT", tag="attnT")
                    nc.scalar.activation(out=attnT, in_=ps_a, func=Exp, scale=SCALE)
                    attn_u = work.tile([P, S], BF16, name="attn_u", tag="attn_u")
                    rowsum = small.tile([P, 1], F32, name="rowsum", tag="rowsum")
                    nc.scalar.activation(
                        out=attn_u, in_=ps_b, func=Exp, scale=SCALE, accum_out=rowsum
                    )
                    for hf in range(2):
                        nc.tensor.matmul(
                            ps_acc[0 : DV + 1, hf * 512 : (hf + 1) * 512],
                            lhsT=v_aug[t][:, pair, sa, :],
                            rhs=attnT[:, hf * 512 : (hf + 1) * 512],
                            start=(t == 0),
                            stop=(t == 7),
                        )
                    recip = small.tile([P, 1], F32, name="recip", tag="recip")
                    nc.vector.reciprocal(out=recip, in_=rowsum)
                    attn_n = work.tile([P, S], BF16, name="attn_n", tag="attn_n")
                    nc.vector.tensor_scalar_mul(attn_n, attn_u, recip)
                    nc.sync.dma_start(
                        out=attns_d[hb, t * P : (t + 1) * P, :], in_=attn_n
                    )

                # ---- normalize PV output: rows 0:64 = outT, row 64 = rowsum
                rsum_sb = norm.tile([P, S], BF16, name="rsum_sb", tag="rsum_sb")
                nc.vector.tensor_copy(
                    out=rsum_sb[DV : DV + 1, :], in_=ps_acc[DV : DV + 1, :]
                )
                ps_bc = psum_b.tile([P, S], F32, name="ps_bc", tag="ps_bb")
                for hf in range(2):
                    nc.tensor.matmul(
                        ps_bc[0:DV, hf * 512 : (hf + 1) * 512],
                        lhsT=ones_t[DV : DV + 1, :],
                        rhs=rsum_sb[DV : DV + 1, hf * 512 : (hf + 1) * 512],
                        start=True,
                        stop=True,
                    )
                recip_bc = norm.tile([P, S], F32, name="recip_bc", tag="recip_bc")
                nc.vector.reciprocal_approx_fast(
                    out=recip_bc[0:DV, :], in_=ps_bc[0:DV, :]
                )
                if sa == 0:
                    nc.vector.tensor_mul(
                        outTs[pair][0:DV, :], ps_acc[0:DV, :], recip_bc[0:DV, :]
                    )
                else:
                    tmp_o = work.tile([DV, S], BF16, name="tmp_o", tag="tmp_o")
                    nc.vector.tensor_mul(tmp_o, ps_acc[0:DV, :], recip_bc[0:DV, :])
                    nc.gpsimd.dma_start(out=outTs[pair][DV:P, :], in_=tmp_o)

        # ---------------- output projection ----------------
        for st in range(8):
            ps_o = psum.tile([P, S], F32, name="ps_o", tag="ps_big")
            for pc in range(NPAIR):
                for hf in range(2):
                    nc.tensor.matmul(
                        ps_o[:, hf * 512 : (hf + 1) * 512],
                        lhsT=outTs[pc][:, st * P : (st + 1) * P],
                        rhs=wp_in[:, pc, hf * 512 : (hf + 1) * 512],
                        start=(pc == 0),
                        stop=(pc == NPAIR - 1),
                    )
            out_sb = work.tile([P, S], F32, name="out_sb", tag="out_sb")
            nc.vector.tensor_copy(out=out_sb, in_=ps_o)
            nc.sync.dma_start(out=out_d[st * P : (st + 1) * P, :], in_=out_sb)

    nc.compile()
    return nc


def _get_nc():
    if "nc" not in _CACHE:
        _CACHE["nc"] = _build_nc()
    return _CACHE["nc"]


def _prep_in_maps(query, key, value, w_q, w_k, w_v, w_proj):
    bf = ml_dtypes.bfloat16
    per_batch = []
    for b in range(B):
        per_batch.append(
            (
                np.ascontiguousarray(query[b].T).astype(bf),
                np.ascontiguousarray(key[b].T).astype(bf),
                np.ascontiguousarray(value[b].T).astype(bf),
            )
        )
    per_half = []
    for half in range(2):
        h0 = half * HHALF
        wq = np.ascontiguousarray(
            w_q[h0 : h0 + HHALF].transpose(1, 0, 2).reshape(D, HHALF * DK)
        ).astype(bf)
        wk = np.ascontiguousarray(
            w_k[h0 : h0 + HHALF].transpose(1, 0, 2).reshape(D, HHALF * DK)
        ).astype(bf)
        wv = np.ascontiguousarray(
            w_v[h0 : h0 + HHALF].transpose(1, 0, 2).reshape(D, HHALF * DV)
        ).astype(bf)
        wpT = np.ascontiguousarray(
            w_proj[:, DV * h0 : DV * (h0 + HHALF)].T
        ).astype(bf)
        per_half.append((wq, wk, wv, wpT))

    in_maps = []
    for core in range(8):
        b, half = divmod(core, 2)
        qT, kT, vT = per_batch[b]
        wq, wk, wv, wpT = per_half[half]
        in_maps.append(
            {"qT": qT, "kT": kT, "vT": vT, "wq": wq, "wk": wk, "wv": wv, "wprojT": wpT}
        )
    return in_maps


def _run(in_maps, trace=False):
    from concourse.bass_utils import run_bass_kernel_spmd

    return run_bass_kernel_spmd(
        _get_nc(), in_maps, core_ids=list(range(8)), trace=trace
    )


def kernel(query, key, value, w_q, w_k, w_v, w_proj, b_proj):
    query = np.asarray(query, dtype=np.float32)
    key = np.asarray(key, dtype=np.float32)
    value = np.asarray(value, dtype=np.float32)
    w_q = np.asarray(w_q, dtype=np.float32)
    w_k = np.asarray(w_k, dtype=np.float32)
    w_v = np.asarray(w_v, dtype=np.float32)
    w_proj = np.asarray(w_proj, dtype=np.float32)
    b_proj = np.asarray(b_proj, dtype=np.float32)

    in_maps = _prep_in_maps(query, key, value, w_q, w_k, w_v, w_proj)
    results = _run(in_maps).results

    out = np.zeros((B, SQ, D), np.float32)
    attns = np.empty((H * B, SQ, SK), np.float32)
    for core in range(8):
        b, half = divmod(core, 2)
        h0 = half * HHALF
        out[b] += results[core]["out"]
        ah = results[core]["attns"].astype(np.float32)
        for i in range(HHALF):
            attns[(h0 + i) * B + b] = ah[i]
    out += b_proj
    return out, attns


# revision 11
# speedup vs baseline: 1.3408x; 1.3408x over previous
"""Trainium2 Bass kernel for nn_MultiHeadAttention (B=4, SQ=SK=1024, D=1024,
H=16, DK=DV=64), sharded over 8 NeuronCores as (batch, head-half).

Each core computes one batch's attention for 8 heads:
  - attnsT_half[h, k, q]  (bf16, TRANSPOSED; host casts + transposes)
  - out_partial[s, D]     (f32; host sums the 2 cores of each batch + b_proj)

Layouts on chip (partition dim first):
  qT/kT/vT   : [D, S] (host-pretransposed, bf16)
  qTs/kTs[p] : [128 = dk(h0)|dk(h1), S]   per head-pair p
  v_aug[st]  : [128 s, 4 pairs, 2 heads, 65 = 64 dv | 1 ones]
  scores     : single [k, q] orientation; the two heads of a pair occupy
               disjoint PE row-groups (dk rows 0:64 / 64:128) and their
               K=64 score matmuls are interleaved so they run concurrently.
  softmax    : exp on ScalarE (scale=1/32 folded in, no max subtraction --
               scores are ~N(0, 0.22), max |score/32| < 3, exp safe);
               rowsum comes from the ones-column of v_aug during PV; it is
               broadcast across partitions with a K=1 ones matmul, inverted
               with reciprocal_approx_fast, and multiplied into attnT (the
               attns output) and the PV result (the out path).
"""

import numpy as np
import ml_dtypes

H, D, DK, DV = 16, 1024, 64, 64
B, SQ, SK = 4, 1024, 1024
P = 128
S = 1024
HHALF = 8  # heads per core
NPAIR = 4  # head pairs per core
SCALE = 1.0 / 32.0  # 1/sqrt(D)

_CACHE = {}


def _build_nc():
    from contextlib import ExitStack

    import concourse.tile as tile
    from concourse import bacc, mybir

    BF16 = mybir.dt.bfloat16
    F32 = mybir.dt.float32
    Exp = mybir.ActivationFunctionType.Exp

    nc = bacc.Bacc("TRN2", target_bir_lowering=False, debug=False)

    qT_d = nc.declare_dram_parameter("qT", [D, S], BF16, isOutput=False)
    kT_d = nc.declare_dram_parameter("kT", [D, S], BF16, isOutput=False)
    vT_d = nc.declare_dram_parameter("vT", [D, S], BF16, isOutput=False)
    wq_d = nc.declare_dram_parameter("wq", [D, HHALF * DK], BF16, isOutput=False)
    wk_d = nc.declare_dram_parameter("wk", [D, HHALF * DK], BF16, isOutput=False)
    wv_d = nc.declare_dram_parameter("wv", [D, HHALF * DV], BF16, isOutput=False)
    wp_d = nc.declare_dram_parameter("wprojT", [HHALF * DV, D], BF16, isOutput=False)
    attns_d = nc.declare_dram_parameter("attnsT", [HHALF, SK, SQ], BF16, isOutput=True)
    out_d = nc.declare_dram_parameter("out", [SQ, D], F32, isOutput=True)

    with ExitStack() as ctx:
        tc = ctx.enter_context(tile.TileContext(nc))
        ins = ctx.enter_context(tc.tile_pool(name="ins", bufs=1))
        proj = ctx.enter_context(tc.tile_pool(name="proj", bufs=1))
        work = ctx.enter_context(tc.tile_pool(name="work", bufs=3))
        atp = ctx.enter_context(tc.tile_pool(name="atp", bufs=18))
        norm = ctx.enter_context(tc.tile_pool(name="norm", bufs=2))
        # PSUM budget: 8 banks of [128, 512]f32.
        #   ps_big ([128,1024] x bufs=2) = 4 banks  (scores / projections / bcast)
        #   pacc   ([128,1024] x bufs=2) = 4 banks  (PV accumulators, one per head)
        psum = ctx.enter_context(tc.tile_pool(name="psum", bufs=2, space="PSUM"))
        pacc = ctx.enter_context(tc.tile_pool(name="pacc", bufs=2, space="PSUM"))

        # ---------------- input loads ----------------
        qT_in = ins.tile([P, 8, S], BF16)
        nc.sync.dma_start(out=qT_in, in_=qT_d.rearrange("(c p) s -> p c s", p=P))
        kT_in = ins.tile([P, 8, S], BF16)
        nc.sync.dma_start(out=kT_in, in_=kT_d.rearrange("(c p) s -> p c s", p=P))
        vT_in = ins.tile([P, 8, S], BF16)
        nc.sync.dma_start(out=vT_in, in_=vT_d.rearrange("(c p) s -> p c s", p=P))
        wq_in = ins.tile([P, 8, HHALF * DK], BF16)
        nc.sync.dma_start(out=wq_in, in_=wq_d.rearrange("(c p) m -> p c m", p=P))
        wk_in = ins.tile([P, 8, HHALF * DK], BF16)
        nc.sync.dma_start(out=wk_in, in_=wk_d.rearrange("(c p) m -> p c m", p=P))
        wv_in = ins.tile([P, 8, HHALF * DV], BF16)
        nc.sync.dma_start(out=wv_in, in_=wv_d.rearrange("(c p) m -> p c m", p=P))
        wp_in = ins.tile([P, NPAIR, D], BF16)
        nc.sync.dma_start(out=wp_in, in_=wp_d.rearrange("(c p) m -> p c m", p=P))

        # ---------------- q/k projections -> [dk-pair, s] ----------------
        # qTs[p][0:64, s]  = q_proj(head 2p) transposed, [64:128, s] = head 2p+1
        qTs, kTs = [], []
        for pair in range(NPAIR):
            for w_in, src_in, dst_list, nm in (
                (wq_in, qT_in, qTs, "q"),
                (wk_in, kT_in, kTs, "k"),
            ):
                dst = proj.tile([P, S], BF16, name=f"{nm}Ts{pair}", tag=f"{nm}Ts{pair}")
                ps = psum.tile([P, S], F32, name=f"ps_{nm}{pair}", tag="ps_big")
                for c in range(8):
                    for hf in range(2):
                        nc.tensor.matmul(
                            ps[:, hf * 512 : (hf + 1) * 512],
                            lhsT=w_in[:, c, pair * P : (pair + 1) * P],
                            rhs=src_in[:, c, hf * 512 : (hf + 1) * 512],
                            start=(c == 0),
                            stop=(c == 7),
                        )
                nc.vector.tensor_copy(out=dst, in_=ps)
                dst_list.append(dst)

        # ---------------- v projection -> [s, head, dv | ones] ----------------
        # v_aug[st] : [128, 4 pairs, 2 heads, 65]; col 64 of each head is 1.0
        v_aug = []
        for st in range(8):
            va = proj.tile([P, NPAIR, 2, DV + 1], BF16, name=f"v_aug{st}", tag=f"v_aug{st}")
            ps = psum.tile([P, HHALF * DV], F32, name=f"ps_v{st}", tag="ps_big")
            for c in range(8):
                nc.tensor.matmul(
                    ps,
                    lhsT=vT_in[:, c, st * P : (st + 1) * P],
                    rhs=wv_in[:, c, :],
                    start=(c == 0),
                    stop=(c == 7),
                )
            ps_v = ps.rearrange("p (pr two d) -> p pr two d", two=2, d=DV)
            nc.vector.tensor_copy(out=va[:, :, :, 0:DV], in_=ps_v)
            nc.vector.memset(va[:, :, :, DV : DV + 1], 1.0)
            v_aug.append(va)

        # ---------------- attention ----------------
        # bf16 ones row at partition 64, used to broadcast the PV rowsum
        # (which lands on partition 64 of the PV accumulator) across all 128
        # partitions via a K=1 matmul (compute engines are lane-local).
        ones_t = proj.tile([P, P], BF16, name="ones_t", tag="ones_t")
        nc.vector.memset(ones_t, 1.0)
        outTs = [
            proj.tile([P, S], BF16, name=f"outTs{p}", tag=f"outTs{p}")
            for p in range(NPAIR)
        ]
        for pair in range(NPAIR):
            accs = [
                pacc.tile([P, S], F32, name=f"ps_acc{sub}", tag="pacc")
                for sub in range(2)
            ]
            atts = [[], []]
            for t in range(8):
                pss = [
                    psum.tile([P, S], F32, name=f"ps_s{sub}", tag="ps_big")
                    for sub in range(2)
                ]
                # Interleave the two heads' K=64 score matmuls: disjoint PE
                # row-groups (dk rows 0:64 vs 64:128) -> concurrent execution.
                for hf in range(2):
                    for sub in range(2):
                        hsl = slice(64 * sub, 64 * sub + 64)
                        nc.tensor.matmul(
                            pss[sub][:, hf * 512 : (hf + 1) * 512],
                            lhsT=kTs[pair][hsl, t * P : (t + 1) * P],
                            rhs=qTs[pair][hsl, hf * 512 : (hf + 1) * 512],
                            start=True,
                            stop=True,
                        )
                for sub in range(2):
                    at = atp.tile([P, S], BF16, name="attnT", tag="attnT")
                    nc.scalar.activation(out=at, in_=pss[sub], func=Exp, scale=SCALE)
                    atts[sub].append(at)
                    for hf in range(2):
                        nc.tensor.matmul(
                            accs[sub][0 : DV + 1, hf * 512 : (hf + 1) * 512],
                            lhsT=v_aug[t][:, pair, sub, :],
                            rhs=at[:, hf * 512 : (hf + 1) * 512],
                            start=(t == 0),
                            stop=(t == 7),
                        )

            for sub in range(2):
                h = 2 * pair + sub
                # rowsum (f32, PSUM row 64) -> bf16 row -> broadcast matmul
                rsum_bf = norm.tile([P, S], BF16, name="rsum_bf", tag="rsum_bf")
                nc.vector.tensor_copy(
                    out=rsum_bf[DV : DV + 1, :], in_=accs[sub][DV : DV + 1, :]
                )
                ps_bc = psum.tile([P, S], F32, name="ps_bc", tag="ps_big")
                for hf in range(2):
                    nc.tensor.matmul(
                        ps_bc[:, hf * 512 : (hf + 1) * 512],
                        lhsT=ones_t[DV : DV + 1, :],
                        rhs=rsum_bf[DV : DV + 1, hf * 512 : (hf + 1) * 512],
                        start=True,
                        stop=True,
                    )
                recip_f = norm.tile([P, S], F32, name="recip_f", tag="recip_f")
                nc.vector.reciprocal_approx_fast(out=recip_f, in_=ps_bc)
                recip_bf = norm.tile([P, S], BF16, name="recip_bf", tag="recip_bf")
                nc.vector.tensor_copy(out=recip_bf, in_=recip_f)

                # normalize attnT in place (bf16 x bf16 -> 2x DVE) and store
                for t in range(8):
                    at = atts[sub][t]
                    nc.vector.tensor_mul(at, at, recip_bf)
                    nc.sync.dma_start(
                        out=attns_d[h, t * P : (t + 1) * P, :], in_=at
                    )

                # normalize PV output rows 0:64 = outT
                if sub == 0:
                    nc.vector.tensor_mul(
                        outTs[pair][0:DV, :], accs[sub][0:DV, :], recip_bf[0:DV, :]
                    )
                else:
                    tmp_o = work.tile([DV, S], BF16, name="tmp_o", tag="tmp_o")
                    nc.vector.tensor_mul(tmp_o, accs[sub][0:DV, :], recip_bf[0:DV, :])
                    nc.gpsimd.dma_start(out=outTs[pair][DV:P, :], in_=tmp_o)

        # ---------------- output projection ----------------
        for st in range(8):
            ps_o = psum.tile([P, S], F32, name="ps_o", tag="ps_big")
            for pc in range(NPAIR):
                for hf in range(2):
                    nc.tensor.matmul(
                        ps_o[:, hf * 512 : (hf + 1) * 512],
                        lhsT=outTs[pc][:, st * P : (st + 1) * P],
                        rhs=wp_in[:, pc, hf * 512 : (hf + 1) * 512],
                        start=(pc == 0),
                        stop=(pc == NPAIR - 1),
                    )
            out_sb = work.tile([P, S], F32, name="out_sb", tag="out_sb")
            nc.vector.tensor_copy(out=out_sb, in_=ps_o)
            nc.sync.dma_start(out=out_d[st * P : (st + 1) * P, :], in_=out_sb)

    nc.compile()
    return nc


def _get_nc():
    if "nc" not in _CACHE:
        _CACHE["nc"] = _build_nc()
    return _CACHE["nc"]


def _prep_in_maps(query, key, value, w_q, w_k, w_v, w_proj):
    bf = ml_dtypes.bfloat16
    per_batch = []
    for b in range(B):
        per_batch.append(
            (
                np.ascontiguousarray(query[b].T).astype(bf),
                np.ascontiguousarray(key[b].T).astype(bf),
                np.ascontiguousarray(value[b].T).astype(bf),
            )
        )
    per_half = []
    for half in range(2):
        h0 = half * HHALF
        wq = np.ascontiguousarray(
            w_q[h0 : h0 + HHALF].transpose(1, 0, 2).reshape(D, HHALF * DK)
        ).astype(bf)
        wk = np.ascontiguousarray(
            w_k[h0 : h0 + HHALF].transpose(1, 0, 2).reshape(D, HHALF * DK)
        ).astype(bf)
        wv = np.ascontiguousarray(
            w_v[h0 : h0 + HHALF].transpose(1, 0, 2).reshape(D, HHALF * DV)
        ).astype(bf)
        wpT = np.ascontiguousarray(
            w_proj[:, DV * h0 : DV * (h0 + HHALF)].T
        ).astype(bf)
        per_half.append((wq, wk, wv, wpT))

    in_maps = []
    for core in range(8):
        b, half = divmod(core, 2)
        qT, kT, vT = per_batch[b]
        wq, wk, wv, wpT = per_half[half]
        in_maps.append(
            {"qT": qT, "kT": kT, "vT": vT, "wq": wq, "wk": wk, "wv": wv, "wprojT": wpT}
        )
    return in_maps


def _run(in_maps, trace=False):
    from concourse.bass_utils import run_bass_kernel_spmd

    return run_bass_kernel_spmd(
        _get_nc(), in_maps, core_ids=list(range(8)), trace=trace
    )


def _gather(results, b_proj):
    out = np.zeros((B, SQ, D), np.float32)
    attns = np.empty((H * B, SQ, SK), np.float32)
    for core in range(8):
        b, half = divmod(core, 2)
        h0 = half * HHALF
        out[b] += results[core]["out"]
        ahT = results[core]["attnsT"]  # [HHALF, SK, SQ] bf16
        for i in range(HHALF):
            attns[(h0 + i) * B + b] = ahT[i].T.astype(np.float32)
    out += b_proj
    return out, attns


def kernel(query, key, value, w_q, w_k, w_v, w_proj, b_proj):
    query = np.asarray(query, dtype=np.float32)
    key = np.asarray(key, dtype=np.float32)
    value = np.asarray(value, dtype=np.float32)
    w_q = np.asarray(w_q, dtype=np.float32)
    w_k = np.asarray(w_k, dtype=np.float32)
    w_v = np.asarray(w_v, dtype=np.float32)
    w_proj = np.asarray(w_proj, dtype=np.float32)
    b_proj = np.asarray(b_proj, dtype=np.float32)

    in_maps = _prep_in_maps(query, key, value, w_q, w_k, w_v, w_proj)
    results = _run(in_maps).results
    return _gather(results, b_proj)


# revision 13
# speedup vs baseline: 1.3999x; 1.0441x over previous
"""Trainium2 Bass kernel for nn_MultiHeadAttention (B=4, SQ=SK=1024, D=1024,
H=16, DK=DV=64), sharded over 8 NeuronCores as (batch, head-half).

Each core computes one batch's attention for 8 heads:
  - attnsT_half[h, k, q]  (bf16, TRANSPOSED; host casts + transposes)
  - out_partial[s, D]     (f32; host sums the 2 cores of each batch + b_proj)

Layouts on chip (partition dim first):
  qT/kT/vT   : [D, S] (host-pretransposed, bf16)
  qTs/kTs[p] : [128 = dk(h0)|dk(h1), S]   per head-pair p
  v_aug[st]  : [128 s, 4 pairs, 2 heads, 65 = 64 dv | 1 ones]
  scores     : single [k, q] orientation; the two heads of a pair occupy
               disjoint PE row-groups (dk rows 0:64 / 64:128) and their
               K=64 score matmuls are interleaved so they run concurrently.
  softmax    : exp on ScalarE (scale=1/32 folded in, no max subtraction --
               scores are ~N(0, 0.22), max |score/32| < 3, exp safe);
               rowsum comes from the ones-column of v_aug during PV; it is
               broadcast across partitions with a K=1 ones matmul, inverted
               with reciprocal_approx_fast, and multiplied into attnT (the
               attns output) and the PV result (the out path).
"""

import numpy as np
import ml_dtypes

H, D, DK, DV = 16, 1024, 64, 64
B, SQ, SK = 4, 1024, 1024
P = 128
S = 1024
HHALF = 8  # heads per core
NPAIR = 4  # head pairs per core
SCALE = 1.0 / 32.0  # 1/sqrt(D)

_CACHE = {}


def _build_nc():
    from contextlib import ExitStack

    import concourse.tile as tile
    from concourse import bacc, mybir

    BF16 = mybir.dt.bfloat16
    F32 = mybir.dt.float32
    Exp = mybir.ActivationFunctionType.Exp

    nc = bacc.Bacc("TRN2", target_bir_lowering=False, debug=False)

    qT_d = nc.declare_dram_parameter("qT", [D, S], BF16, isOutput=False)
    kT_d = nc.declare_dram_parameter("kT", [D, S], BF16, isOutput=False)
    vT_d = nc.declare_dram_parameter("vT", [D, S], BF16, isOutput=False)
    wq_d = nc.declare_dram_parameter("wq", [D, HHALF * DK], BF16, isOutput=False)
    wk_d = nc.declare_dram_parameter("wk", [D, HHALF * DK], BF16, isOutput=False)
    wv_d = nc.declare_dram_parameter("wv", [D, HHALF * DV], BF16, isOutput=False)
    wp_d = nc.declare_dram_parameter("wprojT", [HHALF * DV, D], BF16, isOutput=False)
    attns_d = nc.declare_dram_parameter("attnsT", [HHALF, SK, SQ], BF16, isOutput=True)
    out_d = nc.declare_dram_parameter("out", [SQ, D], F32, isOutput=True)

    with ExitStack() as ctx:
        tc = ctx.enter_context(tile.TileContext(nc))
        ins = ctx.enter_context(tc.tile_pool(name="ins", bufs=1))
        proj = ctx.enter_context(tc.tile_pool(name="proj", bufs=1))
        work = ctx.enter_context(tc.tile_pool(name="work", bufs=3))
        atp = ctx.enter_context(tc.tile_pool(name="atp", bufs=24))
        norm = ctx.enter_context(tc.tile_pool(name="norm", bufs=2))
        # PSUM budget: 8 banks of [128, 512]f32 = one pool of 4 x [128,1024]
        # tiles, shared by projections, scores, PV accumulators, and the
        # rowsum-broadcast matmul.
        psum = ctx.enter_context(tc.tile_pool(name="psum", bufs=4, space="PSUM"))

        # ---------------- input loads ----------------
        qT_in = ins.tile([P, 8, S], BF16)
        nc.sync.dma_start(out=qT_in, in_=qT_d.rearrange("(c p) s -> p c s", p=P))
        kT_in = ins.tile([P, 8, S], BF16)
        nc.sync.dma_start(out=kT_in, in_=kT_d.rearrange("(c p) s -> p c s", p=P))
        vT_in = ins.tile([P, 8, S], BF16)
        nc.sync.dma_start(out=vT_in, in_=vT_d.rearrange("(c p) s -> p c s", p=P))
        wq_in = ins.tile([P, 8, HHALF * DK], BF16)
        nc.sync.dma_start(out=wq_in, in_=wq_d.rearrange("(c p) m -> p c m", p=P))
        wk_in = ins.tile([P, 8, HHALF * DK], BF16)
        nc.sync.dma_start(out=wk_in, in_=wk_d.rearrange("(c p) m -> p c m", p=P))
        wv_in = ins.tile([P, 8, HHALF * DV], BF16)
        nc.sync.dma_start(out=wv_in, in_=wv_d.rearrange("(c p) m -> p c m", p=P))
        wp_in = ins.tile([P, NPAIR, D], BF16)
        nc.sync.dma_start(out=wp_in, in_=wp_d.rearrange("(c p) m -> p c m", p=P))

        # ---------------- q/k projections -> [dk-pair, s] ----------------
        # qTs[p][0:64, s]  = q_proj(head 2p) transposed, [64:128, s] = head 2p+1
        qTs, kTs = [], []
        for pair in range(NPAIR):
            for w_in, src_in, dst_list, nm in (
                (wq_in, qT_in, qTs, "q"),
                (wk_in, kT_in, kTs, "k"),
            ):
                dst = proj.tile([P, S], BF16, name=f"{nm}Ts{pair}", tag=f"{nm}Ts{pair}")
                ps = psum.tile([P, S], F32, name=f"ps_{nm}{pair}", tag="ps_big")
                for c in range(8):
                    for hf in range(2):
                        nc.tensor.matmul(
                            ps[:, hf * 512 : (hf + 1) * 512],
                            lhsT=w_in[:, c, pair * P : (pair + 1) * P],
                            rhs=src_in[:, c, hf * 512 : (hf + 1) * 512],
                            start=(c == 0),
                            stop=(c == 7),
                        )
                nc.vector.tensor_copy(out=dst, in_=ps)
                dst_list.append(dst)

        # ---------------- v projection -> [s, head, dv | ones] ----------------
        # v_aug[st] : [128, 4 pairs, 2 heads, 65]; col 64 of each head is 1.0
        v_aug = []
        for st in range(8):
            va = proj.tile([P, NPAIR, 2, DV + 1], BF16, name=f"v_aug{st}", tag=f"v_aug{st}")
            ps = psum.tile([P, HHALF * DV], F32, name=f"ps_v{st}", tag="ps_big")
            for c in range(8):
                nc.tensor.matmul(
                    ps,
                    lhsT=vT_in[:, c, st * P : (st + 1) * P],
                    rhs=wv_in[:, c, :],
                    start=(c == 0),
                    stop=(c == 7),
                )
            ps_v = ps.rearrange("p (pr two d) -> p pr two d", two=2, d=DV)
            nc.vector.tensor_copy(out=va[:, :, :, 0:DV], in_=ps_v)
            nc.vector.memset(va[:, :, :, DV : DV + 1], 1.0)
            v_aug.append(va)

        # ---------------- attention ----------------
        # bf16 ones row at partition 64, used to broadcast the PV rowsum
        # (which lands on partition 64 of the PV accumulator) across all 128
        # partitions via a K=1 matmul (compute engines are lane-local).
        ones_t = proj.tile([P, P], BF16, name="ones_t", tag="ones_t")
        nc.vector.memset(ones_t, 1.0)
        outTs = [
            proj.tile([P, S], BF16, name=f"outTs{p}", tag=f"outTs{p}")
            for p in range(NPAIR)
        ]
        for pair in range(NPAIR):
            atts = [[], []]
            for t in range(8):
                pss = [
                    psum.tile([P, S], F32, name=f"ps_s{sub}", tag="ps_big")
                    for sub in range(2)
                ]
                # Interleave the two heads' K=64 score matmuls: disjoint PE
                # row-groups (dk rows 0:64 vs 64:128) -> concurrent execution.
                for hf in range(2):
                    for sub in range(2):
                        hsl = slice(64 * sub, 64 * sub + 64)
                        nc.tensor.matmul(
                            pss[sub][:, hf * 512 : (hf + 1) * 512],
                            lhsT=kTs[pair][hsl, t * P : (t + 1) * P],
                            rhs=qTs[pair][hsl, hf * 512 : (hf + 1) * 512],
                            start=True,
                            stop=True,
                        )
                for sub in range(2):
                    at = atp.tile([P, S], BF16, name="attnT", tag="attnT")
                    nc.scalar.activation(out=at, in_=pss[sub], func=Exp, scale=SCALE)
                    atts[sub].append(at)

            # PV: dense full-row matmul bursts, one accumulator per head;
            # overlaps the next pair's ACT-paced score loop.
            accs = [
                psum.tile([P, S], F32, name=f"ps_acc{sub}", tag="ps_big")
                for sub in range(2)
            ]
            for sub in range(2):
                for t in range(8):
                    for hf in range(2):
                        nc.tensor.matmul(
                            accs[sub][0 : DV + 1, hf * 512 : (hf + 1) * 512],
                            lhsT=v_aug[t][:, pair, sub, :],
                            rhs=atts[sub][t][:, hf * 512 : (hf + 1) * 512],
                            start=(t == 0),
                            stop=(t == 7),
                        )

            for sub in range(2):
                h = 2 * pair + sub
                # rowsum (f32, PSUM row 64) -> bf16 row -> broadcast matmul
                rsum_bf = norm.tile([P, S], BF16, name="rsum_bf", tag="rsum_bf")
                nc.vector.tensor_copy(
                    out=rsum_bf[DV : DV + 1, :], in_=accs[sub][DV : DV + 1, :]
                )
                ps_bc = psum.tile([P, S], F32, name="ps_bc", tag="ps_big")
                for hf in range(2):
                    nc.tensor.matmul(
                        ps_bc[:, hf * 512 : (hf + 1) * 512],
                        lhsT=ones_t[DV : DV + 1, :],
                        rhs=rsum_bf[DV : DV + 1, hf * 512 : (hf + 1) * 512],
                        start=True,
                        stop=True,
                    )
                recip_f = norm.tile([P, S], F32, name="recip_f", tag="recip_f")
                nc.vector.reciprocal_approx_fast(out=recip_f, in_=ps_bc)
                recip_bf = norm.tile([P, S], BF16, name="recip_bf", tag="recip_bf")
                nc.vector.tensor_copy(out=recip_bf, in_=recip_f)

                # normalize attnT in place (bf16 x bf16 -> 2x DVE) and store
                for t in range(8):
                    at = atts[sub][t]
                    nc.vector.tensor_mul(at, at, recip_bf)
                    nc.sync.dma_start(
                        out=attns_d[h, t * P : (t + 1) * P, :], in_=at
                    )

                # normalize PV output rows 0:64 = outT
                if sub == 0:
                    nc.vector.tensor_mul(
                        outTs[pair][0:DV, :], accs[sub][0:DV, :], recip_bf[0:DV, :]
                    )
                else:
                    tmp_o = work.tile([DV, S], BF16, name="tmp_o", tag="tmp_o")
                    nc.vector.tensor_mul(tmp_o, accs[sub][0:DV, :], recip_bf[0:DV, :])
                    nc.gpsimd.dma_start(out=outTs[pair][DV:P, :], in_=tmp_o)

        # ---------------- output projection ----------------
        for st in range(8):
            ps_o = psum.tile([P, S], F32, name="ps_o", tag="ps_big")
            for pc in range(NPAIR):
                for hf in range(2):
                    nc.tensor.matmul(
                        ps_o[:, hf * 512 : (hf + 1) * 512],
                        lhsT=outTs[pc][:, st * P : (st + 1) * P],
                        rhs=wp_in[:, pc, hf * 512 : (hf + 1) * 512],
                        start=(pc == 0),
                        stop=(pc == NPAIR - 1),
                    )
            out_sb = work.tile([P, S], F32, name="out_sb", tag="out_sb")
            nc.vector.tensor_copy(out=out_sb, in_=ps_o)
            nc.sync.dma_start(out=out_d[st * P : (st + 1) * P, :], in_=out_sb)

    nc.compile()
    return nc


def _get_nc():
    if "nc" not in _CACHE:
        _CACHE["nc"] = _build_nc()
    return _CACHE["nc"]


def _prep_in_maps(query, key, value, w_q, w_k, w_v, w_proj):
    bf = ml_dtypes.bfloat16
    per_batch = []
    for b in range(B):
        per_batch.append(
            (
                np.ascontiguousarray(query[b].T).astype(bf),
                np.ascontiguousarray(key[b].T).astype(bf),
                np.ascontiguousarray(value[b].T).astype(bf),
            )
        )
    per_half = []
    for half in range(2):
        h0 = half * HHALF
        wq = np.ascontiguousarray(
            w_q[h0 : h0 + HHALF].transpose(1, 0, 2).reshape(D, HHALF * DK)
        ).astype(bf)
        wk = np.ascontiguousarray(
            w_k[h0 : h0 + HHALF].transpose(1, 0, 2).reshape(D, HHALF * DK)
        ).astype(bf)
        wv = np.ascontiguousarray(
            w_v[h0 : h0 + HHALF].transpose(1, 0, 2).reshape(D, HHALF * DV)
        ).astype(bf)
        wpT = np.ascontiguousarray(
            w_proj[:, DV * h0 : DV * (h0 + HHALF)].T
        ).astype(bf)
        per_half.append((wq, wk, wv, wpT))

    in_maps = []
    for core in range(8):
        b, half = divmod(core, 2)
        qT, kT, vT = per_batch[b]
        wq, wk, wv, wpT = per_half[half]
        in_maps.append(
            {"qT": qT, "kT": kT, "vT": vT, "wq": wq, "wk": wk, "wv": wv, "wprojT": wpT}
        )
    return in_maps


def _run(in_maps, trace=False):
    from concourse.bass_utils import run_bass_kernel_spmd

    return run_bass_kernel_spmd(
        _get_nc(), in_maps, core_ids=list(range(8)), trace=trace
    )


def _gather(results, b_proj):
    out = np.zeros((B, SQ, D), np.float32)
    attns = np.empty((H * B, SQ, SK), np.float32)
    for core in range(8):
        b, half = divmod(core, 2)
        h0 = half * HHALF
        out[b] += results[core]["out"]
        ahT = results[core]["attnsT"]  # [HHALF, SK, SQ] bf16
        for i in range(HHALF):
            attns[(h0 + i) * B + b] = ahT[i].T.astype(np.float32)
    out += b_proj
    return out, attns


def kernel(query, key, value, w_q, w_k, w_v, w_proj, b_proj):
    query = np.asarray(query, dtype=np.float32)
    key = np.asarray(key, dtype=np.float32)
    value = np.asarray(value, dtype=np.float32)
    w_q = np.asarray(w_q, dtype=np.float32)
    w_k = np.asarray(w_k, dtype=np.float32)
    w_v = np.asarray(w_v, dtype=np.float32)
    w_proj = np.asarray(w_proj, dtype=np.float32)
    b_proj = np.asarray(b_proj, dtype=np.float32)

    in_maps = _prep_in_maps(query, key, value, w_q, w_k, w_v, w_proj)
    results = _run(in_maps).results
    return _gather(results, b_proj)
